# revision 11
# baseline (speedup 1.0000x reference)
"""GRU-style GNN message-passing kernel for Trainium2 (8 NeuronCores, SPMD).

Reference computation (per node b, features 256, 8 neighbors):
    xr = x @ Wir.T + bir
    hr_n = hs_n @ Whr.T + bhr
    r_n = sigmoid(xr + hr_n)
    z = sigmoid(x @ Wiz.T + biz + h_sum @ Whz.T + bhz)
    s = sum_n r_n * hs_n
    n = tanh(x @ Win.T + bin + s @ Whn.T + bhn)
    out = (1 - z) * n + z * h_sum

Strategy: data-parallel over the node dim B=32768 across 8 cores (4096
rows each), batch-chunked 8x512 per core. Everything on-chip runs in
feature-major ("transposed") layout [256 features = 2 partition chunks
of 128, batch free dim], so every linear layer is a natural PE matmul.
Matmuls and the streamed tensors (x, hs) are bf16 (fp32 PSUM
accumulation); h_sum, the z/n gates and the final combine stay fp32 so
the dominant z*h_sum term keeps fp32-level accuracy.

Per chunk of 512 nodes the schedule keeps the PE stream dense:
  - one PSUM pool, four rotating [128,1024] tiles (all 8 banks):
    allocs xr, r0..r7, n, z per chunk.
  - r neighbors processed in waves of 3 with weight-major matmul
    emission (stationary operand shared across the wave); the shared
    (xr + b_r) term is added into each neighbor's PSUM group via an
    identity matmul.
  - the n-gate's Win@x half and the whole z gate are emitted between
    the r waves and the Whn@s half, giving the PE work while the DVE
    finishes the r*hs product/sum tree.
  - ACT: sigmoid/tanh + the xr bias add; DVE: products + neighbor add
    tree (bf16 2x mode) + 2/3 of the final combine; GPSIMD only does
    d = h - n (fp32), h_sum's bf16 copy is prepared host-side.
"""

import sys
import numpy as np
from contextlib import ExitStack

sys.path.insert(0, "/opt/trn_rl_repo")

import ml_dtypes
import concourse.bacc as bacc
import concourse.tile as tile
from concourse import mybir
from concourse.bass_utils import run_bass_kernel_spmd

F32 = mybir.dt.float32
BF16 = mybir.dt.bfloat16
F8 = mybir.dt.float8e4
BF_NP = ml_dtypes.bfloat16
F8_NP = ml_dtypes.float8_e4m3fn

N_NEIGH, B, IN, H = 8, 32768, 256, 256
M = 8                    # cores
BL = B // M              # rows per core (4096)
NCH = 8                  # batch chunks per core
CW = BL // NCH           # chunk width (512)

_cached = None  # compiled program, reused across kernel() calls


def _build():
    nc = bacc.Bacc("TRN2", target_bir_lowering=False, debug=False, num_devices=M)

    xT = nc.dram_tensor("xT", [IN, BL], BF16, kind="ExternalInput").ap()
    hT = nc.dram_tensor("hT", [H, BL], F32, kind="ExternalInput").ap()
    hbT = nc.dram_tensor("hbT", [H, BL], BF16, kind="ExternalInput").ap()
    hsT = nc.dram_tensor("hsT", [N_NEIGH, H, BL], BF16, kind="ExternalInput").ap()
    # weights packed column-wise: wir[0:512], whr[512:1024], id[1024:1152],
    # wiz[1152:1664], whz[1664:2176], win[2176:2688], whn[2688:3200].
    # Two DMAs: the r-path weights (+id) land first so the PE can start.
    wpack = nc.dram_tensor("wpack", [128, 3200], BF16, kind="ExternalInput").ap()
    biasp = nc.dram_tensor("biasp", [128, 6], F32, kind="ExternalInput").ap()
    outT = nc.dram_tensor("outT", [H, BL], F32, kind="ExternalOutput").ap()

    W_OFF = {"wir": 0, "whr": 512, "wiz": 1152, "whz": 1664,
             "win": 2176, "whn": 2688}
    CWM = CW                      # max chunk width (512); f-halves sit at f*CWM
    widths = [256] + [512] * 6 + [256] * 3

    with tile.TileContext(nc) as tc, ExitStack() as ctx:
        const_pool = ctx.enter_context(tc.tile_pool(name="const", bufs=1))
        x_pool = ctx.enter_context(tc.tile_pool(name="x", bufs=3))
        h_pool = ctx.enter_context(tc.tile_pool(name="h", bufs=3))
        hb_pool = ctx.enter_context(tc.tile_pool(name="hb", bufs=3))
        hs_pool = ctx.enter_context(tc.tile_pool(name="hs", bufs=3))
        xr_pool = ctx.enter_context(tc.tile_pool(name="xr", bufs=2))
        z_pool = ctx.enter_context(tc.tile_pool(name="z", bufs=2))
        s_pool = ctx.enter_context(tc.tile_pool(name="s", bufs=2))
        r_pool = ctx.enter_context(tc.tile_pool(name="r", bufs=2))
        n_pool = ctx.enter_context(tc.tile_pool(name="n", bufs=2))
        d_pool = ctx.enter_context(tc.tile_pool(name="d", bufs=2))
        o_pool = ctx.enter_context(tc.tile_pool(name="o", bufs=2))
        ps_pool = ctx.enter_context(tc.tile_pool(name="ps", bufs=4, space="PSUM"))

        # --- constants ---
        bias_t = const_pool.tile([128, 6], F32, tag="biasp", name="bias_t")
        nc.sync.dma_start(out=bias_t[:, :], in_=biasp[:, :])
        wp_t = const_pool.tile([128, 3200], BF16, tag="wpack", name="wp_t")
        nc.sync.dma_start(out=wp_t[:, 0:512], in_=wpack[:, 0:512])
        nc.sync.dma_start(out=wp_t[:, 512:1152], in_=wpack[:, 512:1152])
        nc.sync.dma_start(out=wp_t[:, 1152:3200], in_=wpack[:, 1152:3200])
        wt = {w: [wp_t[:, o + k * 256:o + (k + 1) * 256] for k in range(2)]
              for w, o in W_OFF.items()}
        id_t = wp_t[:, 1024:1152]
        # warm the ACT spline tables during the initial DMA fill
        warm_t = const_pool.tile([128, 6], F32, tag="warm", name="warm_t")
        nc.scalar.activation(warm_t[:, :], bias_t[:, :],
                             mybir.ActivationFunctionType.Sigmoid)
        nc.scalar.activation(warm_t[:, :], bias_t[:, :],
                             mybir.ActivationFunctionType.Tanh)

        def fcols(t, f):
            return t[:, f * 128:(f + 1) * 128]

        pend = None

        def emit_tail(st, last=False):
            c, sl, cw, pn, sc, zt, ht, nt, dt_, ot = st
            for f in range(2):
                ph = pn[:, f * CWM:f * CWM + cw]
                nc.tensor.matmul(ph, fcols(wt["whn"][0], f),
                                 sc[:, 0:cw], start=False, stop=False)
                nc.tensor.matmul(ph, fcols(wt["whn"][1], f),
                                 sc[:, CWM:CWM + cw], start=False, stop=True)
            for f in range(2):
                nc.scalar.activation(nt[:, f * CWM:f * CWM + cw],
                                     pn[:, f * CWM:f * CWM + cw],
                                     mybir.ActivationFunctionType.Tanh,
                                     bias=bias_t[:, f * 3 + 2:f * 3 + 3])

            def v2(t):  # [128, 2, cw] f-half view
                if cw == CWM:
                    return t[:, 0:2 * CWM]
                return t[:, 0:2 * CWM].rearrange(
                    "p (f b) -> p f b", f=2)[:, :, 0:cw]

            # out = n + z * (h - n); GPSIMD does sub+mul in steady state,
            # the DVE does the whole chain on the last chunk (shorter drain)
            if last:
                nc.vector.tensor_sub(v2(dt_), v2(ht), v2(nt))
                nc.vector.tensor_mul(v2(dt_), v2(zt), v2(dt_))
            else:
                nc.gpsimd.tensor_sub(v2(dt_), v2(ht), v2(nt))
                nc.gpsimd.tensor_mul(v2(dt_), v2(zt), v2(dt_))
            nc.vector.tensor_add(v2(ot), v2(nt), v2(dt_))
            ov = ot[:, 0:2 * CWM].rearrange("p (f b) -> p f b", f=2)
            nc.sync.dma_start(
                out=outT[:, sl].rearrange("(f p) b -> p f b", f=2),
                in_=ov if cw == CWM else ov[:, :, 0:cw])

        off = 0
        for c, cw in enumerate(widths):
            sl = slice(off, off + cw)
            off += cw

            # x first (unblocks the xr matmuls), then the wave-1 hs
            # neighbors, then h/hb, then the rest of hs: per-neighbor DMAs
            # give fine-grained completion so wave 1 starts early.
            def dsl(t, w2):  # dest view: flat-rearranged, cw-sliced if fringe
                v = t[:, 0:w2].rearrange("p (f b) -> p f b", f=2)
                return v if cw == CWM else v[:, :, 0:cw]

            xt = x_pool.tile([128, 2 * CWM], BF16, tag="x", name=f"x_{c}")
            nc.sync.dma_start(
                out=dsl(xt, 2 * CWM),
                in_=xT[:, sl].rearrange("(f p) b -> p f b", f=2))
            hsc = hs_pool.tile([128, 2 * N_NEIGH * CWM], BF16, tag="hs",
                               name=f"hs_{c}")

            def hs_dma(n):
                v = hsc[:, n * 2 * CWM:(n + 1) * 2 * CWM].rearrange(
                    "p (f b) -> p f b", f=2)
                nc.sync.dma_start(
                    out=v if cw == CWM else v[:, :, 0:cw],
                    in_=hsT[n, :, sl].rearrange("(f p) b -> p f b", f=2))

            for n in (0, 1, 2):
                hs_dma(n)
            ht = h_pool.tile([128, 2 * CWM], F32, tag="h", name=f"h_{c}")
            nc.sync.dma_start(
                out=dsl(ht, 2 * CWM),
                in_=hT[:, sl].rearrange("(f p) b -> p f b", f=2))
            htb = hb_pool.tile([128, 2 * CWM], BF16, tag="hb", name=f"hb_{c}")
            nc.sync.dma_start(
                out=dsl(htb, 2 * CWM),
                in_=hbT[:, sl].rearrange("(f p) b -> p f b", f=2))
            for n in (3, 4, 5, 6, 7):
                hs_dma(n)

            def blk(t, b0, nb):  # [128, nb, cw] view of 512-wide blocks
                if cw == CWM:
                    return t[:, b0 * CWM:(b0 + nb) * CWM]
                return t[:, b0 * CWM:(b0 + nb) * CWM].rearrange(
                    "p (k b) -> p k b", k=nb)[:, :, 0:cw]

            def hs_k(n, k):  # [128, cw] matmul operand, k-chunk k
                return hsc[:, (n * 2 + k) * CWM:(n * 2 + k) * CWM + cw]

            # --- PSUM allocation order fixes the slot phase (12 allocs,
            #     4 slots): next chunk's early tiles reuse early-freed slots;
            #     the dummy is never touched so pxr(c+1) recycles instantly ---
            prs = {}
            for n in (0, 1, 2):
                prs[n] = ps_pool.tile([128, 2 * CWM], F32, tag="ps",
                                      name=f"pr{n}_{c}")

            # --- xr = Wir@x (+ b_r via ACT), bf16 in SBUF; later added into
            #     each neighbor's PSUM group via an identity matmul ---
            pxr = ps_pool.tile([128, 2 * CWM], F32, tag="ps", name=f"pxr_{c}")
            for f in range(2):
                ph = pxr[:, f * CWM:f * CWM + cw]
                nc.tensor.matmul(ph, fcols(wt["wir"][0], f), xt[:, 0:cw],
                                 start=True, stop=False)
                nc.tensor.matmul(ph, fcols(wt["wir"][1], f),
                                 xt[:, CWM:CWM + cw], start=False, stop=True)
            xr = xr_pool.tile([128, 2 * CWM], BF16, tag="xr", name=f"xr_{c}")
            for f in range(2):
                nc.scalar.add(xr[:, f * CWM:f * CWM + cw],
                              pxr[:, f * CWM:f * CWM + cw],
                              bias_t[:, f * 3:f * 3 + 1])
            # previous chunk's tail lands here: its Whn@s matmuls sit in the
            # PE queue behind this chunk's xr group, so the PE stays fed
            # while the DVE finishes the previous product/sum tree
            if pend is not None:
                emit_tail(pend)

            # --- r neighbors in waves of 3; weight-major emission inside a
            #     wave so the stationary operand is reused across neighbors ---
            rc = r_pool.tile([128, 2 * N_NEIGH * CWM], BF16, tag="r",
                             name=f"r_{c}")
            for wave in ((0, 1, 2), (3, 4, 5), (6, 7)):
                for n in wave:
                    if n not in prs:
                        prs[n] = ps_pool.tile([128, 2 * CWM], F32, tag="ps",
                                              name=f"pr{n}_{c}")
                for f in range(2):
                    for k in range(2):
                        for n in wave:
                            nc.tensor.matmul(
                                prs[n][:, f * CWM:f * CWM + cw],
                                fcols(wt["whr"][k], f), hs_k(n, k),
                                start=(k == 0), stop=False)
                # identity injection of xr (+b_r); same stationary for all
                for n in wave:
                    for f in range(2):
                        nc.tensor.matmul(prs[n][:, f * CWM:f * CWM + cw],
                                         id_t[:, :],
                                         xr[:, f * CWM:f * CWM + cw],
                                         start=False, stop=True)
                for n in wave:
                    nc.scalar.activation(blk(rc, n * 2, 2), blk(prs[n], 0, 2),
                                         mybir.ActivationFunctionType.Sigmoid)
                # DVE products for completed sigmoids, in place in hsc;
                # wide block adds; short post-r7 chain (mul6 under r7's ACT)
                with nc.allow_low_precision(reason="bf16 neighbor prod/sums"):
                    if wave[0] == 0:
                        nc.vector.tensor_mul(blk(hsc, 0, 4), blk(rc, 0, 4),
                                             blk(hsc, 0, 4))
                    elif wave[0] == 3:
                        nc.vector.tensor_mul(blk(hsc, 4, 8), blk(rc, 4, 8),
                                             blk(hsc, 4, 8))
                        # (n0,n1) += (n2,n3), then n0 += n1
                        nc.vector.tensor_add(blk(hsc, 0, 4), blk(hsc, 0, 4),
                                             blk(hsc, 4, 4))
                        nc.vector.tensor_add(blk(hsc, 0, 2), blk(hsc, 0, 2),
                                             blk(hsc, 2, 2))
                    else:
                        nc.vector.tensor_add(blk(hsc, 8, 2), blk(hsc, 8, 2),
                                             blk(hsc, 10, 2))
                        nc.vector.tensor_mul(blk(hsc, 12, 2), blk(rc, 12, 2),
                                             blk(hsc, 12, 2))
                        nc.vector.tensor_mul(blk(hsc, 14, 2), blk(rc, 14, 2),
                                             blk(hsc, 14, 2))
                        nc.vector.tensor_add(blk(hsc, 12, 2), blk(hsc, 12, 2),
                                             blk(hsc, 14, 2))
                        nc.vector.tensor_add(blk(hsc, 8, 2), blk(hsc, 8, 2),
                                             blk(hsc, 12, 2))

            sc = s_pool.tile([128, 2 * CWM], BF16, tag="s", name=f"s_{c}")
            with nc.allow_low_precision(reason="bf16 neighbor sums"):
                nc.vector.tensor_add(blk(sc, 0, 2), blk(hsc, 0, 2),
                                     blk(hsc, 8, 2))

            # --- n gate part 1: Win@x into PSUM (keeps PE busy while the
            #     DVE finishes the tree), then the whole z gate; Whn@s and
            #     everything after it are deferred into the next chunk ---
            pn = ps_pool.tile([128, 2 * CWM], F32, tag="ps", name=f"pn_{c}")
            for f in range(2):
                ph = pn[:, f * CWM:f * CWM + cw]
                nc.tensor.matmul(ph, fcols(wt["win"][0], f), xt[:, 0:cw],
                                 start=True, stop=False)
                nc.tensor.matmul(ph, fcols(wt["win"][1], f),
                                 xt[:, CWM:CWM + cw], start=False, stop=False)

            pz = ps_pool.tile([128, 2 * CWM], F32, tag="ps", name=f"pz_{c}")
            ps_pool.tile([128, 2 * CWM], F32, tag="ps", name=f"dummy_{c}")
            for f in range(2):
                ph = pz[:, f * CWM:f * CWM + cw]
                nc.tensor.matmul(ph, fcols(wt["wiz"][0], f), xt[:, 0:cw],
                                 start=True, stop=False)
                nc.tensor.matmul(ph, fcols(wt["wiz"][1], f),
                                 xt[:, CWM:CWM + cw], start=False, stop=False)
                nc.tensor.matmul(ph, fcols(wt["whz"][0], f), htb[:, 0:cw],
                                 start=False, stop=False)
                nc.tensor.matmul(ph, fcols(wt["whz"][1], f),
                                 htb[:, CWM:CWM + cw], start=False, stop=True)
            zt = z_pool.tile([128, 2 * CWM], F32, tag="z", name=f"z_{c}")
            for f in range(2):
                nc.scalar.activation(zt[:, f * CWM:f * CWM + cw],
                                     pz[:, f * CWM:f * CWM + cw],
                                     mybir.ActivationFunctionType.Sigmoid,
                                     bias=bias_t[:, f * 3 + 1:f * 3 + 2])

            nt = n_pool.tile([128, 2 * CWM], F32, tag="n", name=f"n_{c}")
            dt_ = d_pool.tile([128, 2 * CWM], F32, tag="d", name=f"d_{c}")
            ot = o_pool.tile([128, 2 * CWM], F32, tag="o", name=f"o_{c}")
            pend = (c, sl, cw, pn, sc, zt, ht, nt, dt_, ot)

        emit_tail(pend, last=True)

    nc.compile()
    return nc


def _prep_inputs(x, h_sum, hs, Wir, bir, Whr, bhr, Wiz, biz, Whz, bhz,
                 Win, bin_, Whn, bhn):
    """Shard + transpose to feature-major per-core input maps."""
    f32 = np.float32
    xT = np.asarray(x, f32).T.astype(BF_NP)                  # [256, B] bf16
    hT = np.ascontiguousarray(np.asarray(h_sum, f32).T)      # [256, B] f32
    hbT = hT.astype(BF_NP)                                   # [256, B] bf16
    hsT = np.asarray(hs, f32).transpose(0, 2, 1).astype(BF_NP)  # [8,256,B] bf16

    wpack = np.empty((128, 3200), BF_NP)
    offs = {"wir": 0, "whr": 512, "wiz": 1152, "whz": 1664,
            "win": 2176, "whn": 2688}
    mats = {"wir": Wir, "whr": Whr, "wiz": Wiz, "whz": Whz,
            "win": Win, "whn": Whn}
    for wname, o in offs.items():
        wT = np.asarray(mats[wname], f32).T.astype(BF_NP)   # [256 in, 256 out]
        for k in range(2):
            wpack[:, o + k * 256:o + (k + 1) * 256] = \
                wT[k * 128:(k + 1) * 128, :]
    wpack[:, 1024:1152] = np.eye(128, dtype=f32).astype(BF_NP)
    b_r = np.asarray(bir, f32) + np.asarray(bhr, f32)
    b_z = np.asarray(biz, f32) + np.asarray(bhz, f32)
    b_n = np.asarray(bin_, f32) + np.asarray(bhn, f32)
    biasp = np.empty((128, 6), f32)
    for f in range(2):
        biasp[:, f * 3 + 0] = b_r[f * 128:(f + 1) * 128]
        biasp[:, f * 3 + 1] = b_z[f * 128:(f + 1) * 128]
        biasp[:, f * 3 + 2] = b_n[f * 128:(f + 1) * 128]

    in_maps = []
    for c in range(M):
        sl = slice(c * BL, (c + 1) * BL)
        m = {
            "xT": np.ascontiguousarray(xT[:, sl]),
            "hT": np.ascontiguousarray(hT[:, sl]),
            "hbT": np.ascontiguousarray(hbT[:, sl]),
            "hsT": np.ascontiguousarray(hsT[:, :, sl]),
            "wpack": wpack,
            "biasp": biasp,
        }
        in_maps.append(m)
    return in_maps


def _run(inputs, trace=False, **trace_kwargs):
    global _cached
    if _cached is None:
        _cached = _build()
    nc = _cached
    in_maps = _prep_inputs(**inputs)
    res = run_bass_kernel_spmd(nc, in_maps, list(range(M)), trace=trace,
                               **trace_kwargs)
    out = np.empty((B, H), np.float32)
    for c in range(M):
        out[c * BL:(c + 1) * BL, :] = res.results[c]["outT"].T
    return out, res


def kernel(**inputs):
    return _run(inputs)[0]


# revision 12
# speedup vs baseline: 1.0299x; 1.0299x over previous
"""GRU-style GNN message-passing kernel for Trainium2 (8 NeuronCores, SPMD).

Reference computation (per node b, features 256, 8 neighbors):
    xr = x @ Wir.T + bir
    hr_n = hs_n @ Whr.T + bhr
    r_n = sigmoid(xr + hr_n)
    z = sigmoid(x @ Wiz.T + biz + h_sum @ Whz.T + bhz)
    s = sum_n r_n * hs_n
    n = tanh(x @ Win.T + bin + s @ Whn.T + bhn)
    out = (1 - z) * n + z * h_sum

Strategy: data-parallel over the node dim B=32768 across 8 cores (4096
rows each), batch-chunked 8x512 per core. Everything on-chip runs in
feature-major ("transposed") layout [256 features = 2 partition chunks
of 128, batch free dim], so every linear layer is a natural PE matmul.
Matmuls and the streamed tensors (x, hs) are bf16 (fp32 PSUM
accumulation); h_sum, the z/n gates and the final combine stay fp32 so
the dominant z*h_sum term keeps fp32-level accuracy.

Per chunk of 512 nodes the schedule keeps the PE stream dense:
  - one PSUM pool, four rotating [128,1024] tiles (all 8 banks):
    allocs xr, r0..r7, n, z per chunk.
  - r neighbors processed in waves of 3 with weight-major matmul
    emission (stationary operand shared across the wave); the shared
    (xr + b_r) term is added into each neighbor's PSUM group via an
    identity matmul.
  - the n-gate's Win@x half and the whole z gate are emitted between
    the r waves and the Whn@s half, giving the PE work while the DVE
    finishes the r*hs product/sum tree.
  - ACT: sigmoid/tanh + the xr bias add; DVE: products + neighbor add
    tree (bf16 2x mode) + 2/3 of the final combine; GPSIMD only does
    d = h - n (fp32), h_sum's bf16 copy is prepared host-side.
"""

import sys
import numpy as np
from contextlib import ExitStack

sys.path.insert(0, "/opt/trn_rl_repo")

import ml_dtypes
import concourse.bacc as bacc
import concourse.tile as tile
from concourse import mybir
from concourse.bass_utils import run_bass_kernel_spmd

F32 = mybir.dt.float32
BF16 = mybir.dt.bfloat16
F8 = mybir.dt.float8e4
BF_NP = ml_dtypes.bfloat16
F8_NP = ml_dtypes.float8_e4m3fn

N_NEIGH, B, IN, H = 8, 32768, 256, 256
M = 8                    # cores
BL = B // M              # rows per core (4096)
NCH = 8                  # batch chunks per core
CW = BL // NCH           # chunk width (512)

_cached = None  # compiled program, reused across kernel() calls


def _build():
    nc = bacc.Bacc("TRN2", target_bir_lowering=False, debug=False, num_devices=M)

    xT = nc.dram_tensor("xT", [IN, BL], BF16, kind="ExternalInput").ap()
    hT = nc.dram_tensor("hT", [H, BL], F32, kind="ExternalInput").ap()
    hbT = nc.dram_tensor("hbT", [H, BL], BF16, kind="ExternalInput").ap()
    hsT = nc.dram_tensor("hsT", [N_NEIGH, H, BL], BF16, kind="ExternalInput").ap()
    # weights packed column-wise: wir[0:512], whr[512:1024], id[1024:1152],
    # wiz[1152:1664], whz[1664:2176], win[2176:2688], whn[2688:3200].
    # Two DMAs: the r-path weights (+id) land first so the PE can start.
    wpack = nc.dram_tensor("wpack", [128, 3200], BF16, kind="ExternalInput").ap()
    biasp = nc.dram_tensor("biasp", [128, 6], F32, kind="ExternalInput").ap()
    outT = nc.dram_tensor("outT", [H, BL], F32, kind="ExternalOutput").ap()

    W_OFF = {"wir": 0, "whr": 512, "wiz": 1152, "whz": 1664,
             "win": 2176, "whn": 2688}
    CWM = CW                      # max chunk width (512); f-halves sit at f*CWM
    widths = [256] + [512] * 7 + [256]

    with tile.TileContext(nc) as tc, ExitStack() as ctx:
        const_pool = ctx.enter_context(tc.tile_pool(name="const", bufs=1))
        x_pool = ctx.enter_context(tc.tile_pool(name="x", bufs=3))
        h_pool = ctx.enter_context(tc.tile_pool(name="h", bufs=3))
        hb_pool = ctx.enter_context(tc.tile_pool(name="hb", bufs=3))
        hs_pool = ctx.enter_context(tc.tile_pool(name="hs", bufs=3))
        xr_pool = ctx.enter_context(tc.tile_pool(name="xr", bufs=2))
        z_pool = ctx.enter_context(tc.tile_pool(name="z", bufs=2))
        s_pool = ctx.enter_context(tc.tile_pool(name="s", bufs=2))
        r_pool = ctx.enter_context(tc.tile_pool(name="r", bufs=2))
        n_pool = ctx.enter_context(tc.tile_pool(name="n", bufs=2))
        d_pool = ctx.enter_context(tc.tile_pool(name="d", bufs=2))
        o_pool = ctx.enter_context(tc.tile_pool(name="o", bufs=2))
        ps_pool = ctx.enter_context(tc.tile_pool(name="ps", bufs=4, space="PSUM"))

        # --- constants ---
        bias_t = const_pool.tile([128, 6], F32, tag="biasp", name="bias_t")
        nc.sync.dma_start(out=bias_t[:, :], in_=biasp[:, :])
        wp_t = const_pool.tile([128, 3200], BF16, tag="wpack", name="wp_t")
        nc.sync.dma_start(out=wp_t[:, 0:512], in_=wpack[:, 0:512])
        nc.sync.dma_start(out=wp_t[:, 512:1152], in_=wpack[:, 512:1152])
        nc.sync.dma_start(out=wp_t[:, 1152:3200], in_=wpack[:, 1152:3200])
        wt = {w: [wp_t[:, o + k * 256:o + (k + 1) * 256] for k in range(2)]
              for w, o in W_OFF.items()}
        id_t = wp_t[:, 1024:1152]
        # warm the ACT spline tables during the initial DMA fill
        warm_t = const_pool.tile([128, 6], F32, tag="warm", name="warm_t")
        nc.scalar.activation(warm_t[:, :], bias_t[:, :],
                             mybir.ActivationFunctionType.Sigmoid)
        nc.scalar.activation(warm_t[:, :], bias_t[:, :],
                             mybir.ActivationFunctionType.Tanh)

        def fcols(t, f):
            return t[:, f * 128:(f + 1) * 128]

        pend = None

        def emit_tail(st, last=False):
            c, sl, cw, pn, sc, zt, ht, nt, dt_, ot = st
            for f in range(2):
                ph = pn[:, f * CWM:f * CWM + cw]
                nc.tensor.matmul(ph, fcols(wt["whn"][0], f),
                                 sc[:, 0:cw], start=False, stop=False)
                nc.tensor.matmul(ph, fcols(wt["whn"][1], f),
                                 sc[:, CWM:CWM + cw], start=False, stop=True)
            for f in range(2):
                nc.scalar.activation(nt[:, f * CWM:f * CWM + cw],
                                     pn[:, f * CWM:f * CWM + cw],
                                     mybir.ActivationFunctionType.Tanh,
                                     bias=bias_t[:, f * 3 + 2:f * 3 + 3])

            def v2(t):  # [128, 2, cw] f-half view
                if cw == CWM:
                    return t[:, 0:2 * CWM]
                return t[:, 0:2 * CWM].rearrange(
                    "p (f b) -> p f b", f=2)[:, :, 0:cw]

            # out = n + z * (h - n); GPSIMD does sub+mul in steady state,
            # the DVE does the whole chain on the last chunk (shorter drain)
            if last:
                nc.vector.tensor_sub(v2(dt_), v2(ht), v2(nt))
                nc.vector.tensor_mul(v2(dt_), v2(zt), v2(dt_))
            else:
                nc.gpsimd.tensor_sub(v2(dt_), v2(ht), v2(nt))
                nc.gpsimd.tensor_mul(v2(dt_), v2(zt), v2(dt_))
            nc.vector.tensor_add(v2(ot), v2(nt), v2(dt_))
            ov = ot[:, 0:2 * CWM].rearrange("p (f b) -> p f b", f=2)
            nc.sync.dma_start(
                out=outT[:, sl].rearrange("(f p) b -> p f b", f=2),
                in_=ov if cw == CWM else ov[:, :, 0:cw])

        off = 0
        for c, cw in enumerate(widths):
            sl = slice(off, off + cw)
            off += cw

            # x first (unblocks the xr matmuls), then the wave-1 hs
            # neighbors, then h/hb, then the rest of hs: per-neighbor DMAs
            # give fine-grained completion so wave 1 starts early.
            def dsl(t, w2):  # dest view: flat-rearranged, cw-sliced if fringe
                v = t[:, 0:w2].rearrange("p (f b) -> p f b", f=2)
                return v if cw == CWM else v[:, :, 0:cw]

            xt = x_pool.tile([128, 2 * CWM], BF16, tag="x", name=f"x_{c}")
            nc.sync.dma_start(
                out=dsl(xt, 2 * CWM),
                in_=xT[:, sl].rearrange("(f p) b -> p f b", f=2))
            hsc = hs_pool.tile([128, 2 * N_NEIGH * CWM], BF16, tag="hs",
                               name=f"hs_{c}")

            def hs_dma(n):
                v = hsc[:, n * 2 * CWM:(n + 1) * 2 * CWM].rearrange(
                    "p (f b) -> p f b", f=2)
                nc.sync.dma_start(
                    out=v if cw == CWM else v[:, :, 0:cw],
                    in_=hsT[n, :, sl].rearrange("(f p) b -> p f b", f=2))

            for n in (0, 1, 2):
                hs_dma(n)
            ht = h_pool.tile([128, 2 * CWM], F32, tag="h", name=f"h_{c}")
            nc.sync.dma_start(
                out=dsl(ht, 2 * CWM),
                in_=hT[:, sl].rearrange("(f p) b -> p f b", f=2))
            htb = hb_pool.tile([128, 2 * CWM], BF16, tag="hb", name=f"hb_{c}")
            nc.sync.dma_start(
                out=dsl(htb, 2 * CWM),
                in_=hbT[:, sl].rearrange("(f p) b -> p f b", f=2))
            for n in (3, 4, 5, 6, 7):
                hs_dma(n)

            def blk(t, b0, nb):  # [128, nb, cw] view of 512-wide blocks
                if cw == CWM:
                    return t[:, b0 * CWM:(b0 + nb) * CWM]
                return t[:, b0 * CWM:(b0 + nb) * CWM].rearrange(
                    "p (k b) -> p k b", k=nb)[:, :, 0:cw]

            def hs_k(n, k):  # [128, cw] matmul operand, k-chunk k
                return hsc[:, (n * 2 + k) * CWM:(n * 2 + k) * CWM + cw]

            # --- xr = Wir@x (+ b_r via ACT), bf16 in SBUF; later added into
            #     each neighbor's PSUM group via an identity matmul ---
            pxr = ps_pool.tile([128, 2 * CWM], F32, tag="ps", name=f"pxr_{c}")
            for f in range(2):
                ph = pxr[:, f * CWM:f * CWM + cw]
                nc.tensor.matmul(ph, fcols(wt["wir"][0], f), xt[:, 0:cw],
                                 start=True, stop=False)
                nc.tensor.matmul(ph, fcols(wt["wir"][1], f),
                                 xt[:, CWM:CWM + cw], start=False, stop=True)
            xr = xr_pool.tile([128, 2 * CWM], BF16, tag="xr", name=f"xr_{c}")
            for f in range(2):
                nc.scalar.add(xr[:, f * CWM:f * CWM + cw],
                              pxr[:, f * CWM:f * CWM + cw],
                              bias_t[:, f * 3:f * 3 + 1])
            # previous chunk's tail lands here: its Whn@s matmuls sit in the
            # PE queue behind this chunk's xr group, so the PE stays fed
            # while the DVE finishes the previous product/sum tree
            if pend is not None:
                emit_tail(pend)

            # --- r neighbors in waves of 3; weight-major emission inside a
            #     wave so the stationary operand is reused across neighbors ---
            rc = r_pool.tile([128, 2 * N_NEIGH * CWM], BF16, tag="r",
                             name=f"r_{c}")
            prs = {}
            for wave in ((0, 1, 2), (3, 4, 5), (6, 7)):
                for n in wave:
                    prs[n] = ps_pool.tile([128, 2 * CWM], F32, tag="ps",
                                          name=f"pr{n}_{c}")
                for f in range(2):
                    for k in range(2):
                        for n in wave:
                            nc.tensor.matmul(
                                prs[n][:, f * CWM:f * CWM + cw],
                                fcols(wt["whr"][k], f), hs_k(n, k),
                                start=(k == 0), stop=False)
                # identity injection of xr (+b_r); same stationary for all
                for n in wave:
                    for f in range(2):
                        nc.tensor.matmul(prs[n][:, f * CWM:f * CWM + cw],
                                         id_t[:, :],
                                         xr[:, f * CWM:f * CWM + cw],
                                         start=False, stop=True)
                for n in wave:
                    nc.scalar.activation(blk(rc, n * 2, 2), blk(prs[n], 0, 2),
                                         mybir.ActivationFunctionType.Sigmoid)
                # DVE products for completed sigmoids, in place in hsc;
                # wide block adds; short post-r7 chain (mul6 under r7's ACT)
                with nc.allow_low_precision(reason="bf16 neighbor prod/sums"):
                    if wave[0] == 0:
                        nc.vector.tensor_mul(blk(hsc, 0, 4), blk(rc, 0, 4),
                                             blk(hsc, 0, 4))
                    elif wave[0] == 3:
                        nc.vector.tensor_mul(blk(hsc, 4, 8), blk(rc, 4, 8),
                                             blk(hsc, 4, 8))
                        # (n0,n1) += (n2,n3), then n0 += n1
                        nc.vector.tensor_add(blk(hsc, 0, 4), blk(hsc, 0, 4),
                                             blk(hsc, 4, 4))
                        nc.vector.tensor_add(blk(hsc, 0, 2), blk(hsc, 0, 2),
                                             blk(hsc, 2, 2))
                    else:
                        # fold n4,n5 into n0 before r7's sigmoid lands: the
                        # post-r7 chain is just mul7 + (n6+=n7) + final sc
                        nc.vector.tensor_add(blk(hsc, 8, 2), blk(hsc, 8, 2),
                                             blk(hsc, 10, 2))
                        nc.vector.tensor_add(blk(hsc, 0, 2), blk(hsc, 0, 2),
                                             blk(hsc, 8, 2))
                        nc.vector.tensor_mul(blk(hsc, 12, 2), blk(rc, 12, 2),
                                             blk(hsc, 12, 2))
                        nc.vector.tensor_mul(blk(hsc, 14, 2), blk(rc, 14, 2),
                                             blk(hsc, 14, 2))
                        nc.vector.tensor_add(blk(hsc, 12, 2), blk(hsc, 12, 2),
                                             blk(hsc, 14, 2))

            sc = s_pool.tile([128, 2 * CWM], BF16, tag="s", name=f"s_{c}")
            with nc.allow_low_precision(reason="bf16 neighbor sums"):
                nc.vector.tensor_add(blk(sc, 0, 2), blk(hsc, 0, 2),
                                     blk(hsc, 12, 2))

            # --- n gate part 1: Win@x into PSUM (keeps PE busy while the
            #     DVE finishes the tree), then the whole z gate; Whn@s and
            #     everything after it are deferred into the next chunk ---
            pn = ps_pool.tile([128, 2 * CWM], F32, tag="ps", name=f"pn_{c}")
            for f in range(2):
                ph = pn[:, f * CWM:f * CWM + cw]
                nc.tensor.matmul(ph, fcols(wt["win"][0], f), xt[:, 0:cw],
                                 start=True, stop=False)
                nc.tensor.matmul(ph, fcols(wt["win"][1], f),
                                 xt[:, CWM:CWM + cw], start=False, stop=False)

            pz = ps_pool.tile([128, 2 * CWM], F32, tag="ps", name=f"pz_{c}")
            for f in range(2):
                ph = pz[:, f * CWM:f * CWM + cw]
                nc.tensor.matmul(ph, fcols(wt["wiz"][0], f), xt[:, 0:cw],
                                 start=True, stop=False)
                nc.tensor.matmul(ph, fcols(wt["wiz"][1], f),
                                 xt[:, CWM:CWM + cw], start=False, stop=False)
                nc.tensor.matmul(ph, fcols(wt["whz"][0], f), htb[:, 0:cw],
                                 start=False, stop=False)
                nc.tensor.matmul(ph, fcols(wt["whz"][1], f),
                                 htb[:, CWM:CWM + cw], start=False, stop=True)
            zt = z_pool.tile([128, 2 * CWM], F32, tag="z", name=f"z_{c}")
            for f in range(2):
                nc.scalar.activation(zt[:, f * CWM:f * CWM + cw],
                                     pz[:, f * CWM:f * CWM + cw],
                                     mybir.ActivationFunctionType.Sigmoid,
                                     bias=bias_t[:, f * 3 + 1:f * 3 + 2])

            nt = n_pool.tile([128, 2 * CWM], F32, tag="n", name=f"n_{c}")
            dt_ = d_pool.tile([128, 2 * CWM], F32, tag="d", name=f"d_{c}")
            ot = o_pool.tile([128, 2 * CWM], F32, tag="o", name=f"o_{c}")
            pend = (c, sl, cw, pn, sc, zt, ht, nt, dt_, ot)

        emit_tail(pend, last=True)

    nc.compile()
    return nc


def _prep_inputs(x, h_sum, hs, Wir, bir, Whr, bhr, Wiz, biz, Whz, bhz,
                 Win, bin_, Whn, bhn):
    """Shard + transpose to feature-major per-core input maps."""
    f32 = np.float32
    xT = np.asarray(x, f32).T.astype(BF_NP)                  # [256, B] bf16
    hT = np.ascontiguousarray(np.asarray(h_sum, f32).T)      # [256, B] f32
    hbT = hT.astype(BF_NP)                                   # [256, B] bf16
    hsT = np.asarray(hs, f32).transpose(0, 2, 1).astype(BF_NP)  # [8,256,B] bf16

    wpack = np.empty((128, 3200), BF_NP)
    offs = {"wir": 0, "whr": 512, "wiz": 1152, "whz": 1664,
            "win": 2176, "whn": 2688}
    mats = {"wir": Wir, "whr": Whr, "wiz": Wiz, "whz": Whz,
            "win": Win, "whn": Whn}
    for wname, o in offs.items():
        wT = np.asarray(mats[wname], f32).T.astype(BF_NP)   # [256 in, 256 out]
        for k in range(2):
            wpack[:, o + k * 256:o + (k + 1) * 256] = \
                wT[k * 128:(k + 1) * 128, :]
    wpack[:, 1024:1152] = np.eye(128, dtype=f32).astype(BF_NP)
    b_r = np.asarray(bir, f32) + np.asarray(bhr, f32)
    b_z = np.asarray(biz, f32) + np.asarray(bhz, f32)
    b_n = np.asarray(bin_, f32) + np.asarray(bhn, f32)
    biasp = np.empty((128, 6), f32)
    for f in range(2):
        biasp[:, f * 3 + 0] = b_r[f * 128:(f + 1) * 128]
        biasp[:, f * 3 + 1] = b_z[f * 128:(f + 1) * 128]
        biasp[:, f * 3 + 2] = b_n[f * 128:(f + 1) * 128]

    in_maps = []
    for c in range(M):
        sl = slice(c * BL, (c + 1) * BL)
        m = {
            "xT": np.ascontiguousarray(xT[:, sl]),
            "hT": np.ascontiguousarray(hT[:, sl]),
            "hbT": np.ascontiguousarray(hbT[:, sl]),
            "hsT": np.ascontiguousarray(hsT[:, :, sl]),
            "wpack": wpack,
            "biasp": biasp,
        }
        in_maps.append(m)
    return in_maps


def _run(inputs, trace=False, **trace_kwargs):
    global _cached
    if _cached is None:
        _cached = _build()
    nc = _cached
    in_maps = _prep_inputs(**inputs)
    res = run_bass_kernel_spmd(nc, in_maps, list(range(M)), trace=trace,
                               **trace_kwargs)
    out = np.empty((B, H), np.float32)
    for c in range(M):
        out[c * BL:(c + 1) * BL, :] = res.results[c]["outT"].T
    return out, res


def kernel(**inputs):
    return _run(inputs)[0]


# revision 13
# speedup vs baseline: 1.0377x; 1.0076x over previous
"""GRU-style GNN message-passing kernel for Trainium2 (8 NeuronCores, SPMD).

Reference computation (per node b, features 256, 8 neighbors):
    xr = x @ Wir.T + bir
    hr_n = hs_n @ Whr.T + bhr
    r_n = sigmoid(xr + hr_n)
    z = sigmoid(x @ Wiz.T + biz + h_sum @ Whz.T + bhz)
    s = sum_n r_n * hs_n
    n = tanh(x @ Win.T + bin + s @ Whn.T + bhn)
    out = (1 - z) * n + z * h_sum

Strategy: data-parallel over the node dim B=32768 across 8 cores (4096
rows each), batch-chunked 8x512 per core. Everything on-chip runs in
feature-major ("transposed") layout [256 features = 2 partition chunks
of 128, batch free dim], so every linear layer is a natural PE matmul.
Matmuls and the streamed tensors (x, hs) are bf16 (fp32 PSUM
accumulation); h_sum, the z/n gates and the final combine stay fp32 so
the dominant z*h_sum term keeps fp32-level accuracy.

Per chunk of 512 nodes the schedule keeps the PE stream dense:
  - one PSUM pool, four rotating [128,1024] tiles (all 8 banks):
    allocs xr, r0..r7, n, z per chunk.
  - r neighbors processed in waves of 3 with weight-major matmul
    emission (stationary operand shared across the wave); the shared
    (xr + b_r) term is added into each neighbor's PSUM group via an
    identity matmul.
  - the n-gate's Win@x half and the whole z gate are emitted between
    the r waves and the Whn@s half, giving the PE work while the DVE
    finishes the r*hs product/sum tree.
  - ACT: sigmoid/tanh + the xr bias add; DVE: products + neighbor add
    tree (bf16 2x mode) + 2/3 of the final combine; GPSIMD only does
    d = h - n (fp32), h_sum's bf16 copy is prepared host-side.
"""

import sys
import numpy as np
from contextlib import ExitStack

sys.path.insert(0, "/opt/trn_rl_repo")

import ml_dtypes
import concourse.bacc as bacc
import concourse.tile as tile
from concourse import mybir
from concourse.bass_utils import run_bass_kernel_spmd

F32 = mybir.dt.float32
BF16 = mybir.dt.bfloat16
F8 = mybir.dt.float8e4
BF_NP = ml_dtypes.bfloat16
F8_NP = ml_dtypes.float8_e4m3fn

N_NEIGH, B, IN, H = 8, 32768, 256, 256
M = 8                    # cores
BL = B // M              # rows per core (4096)
NCH = 8                  # batch chunks per core
CW = BL // NCH           # chunk width (512)

_cached = None  # compiled program, reused across kernel() calls


def _build():
    nc = bacc.Bacc("TRN2", target_bir_lowering=False, debug=False, num_devices=M)

    xT = nc.dram_tensor("xT", [IN, BL], BF16, kind="ExternalInput").ap()
    hT = nc.dram_tensor("hT", [H, BL], F32, kind="ExternalInput").ap()
    hbT = nc.dram_tensor("hbT", [H, BL], BF16, kind="ExternalInput").ap()
    hsT = nc.dram_tensor("hsT", [N_NEIGH, H, BL], BF16, kind="ExternalInput").ap()
    # weights packed column-wise: wir[0:512], whr[512:1024], id[1024:1152],
    # wiz[1152:1664], whz[1664:2176], win[2176:2688], whn[2688:3200].
    # Two DMAs: the r-path weights (+id) land first so the PE can start.
    wpack = nc.dram_tensor("wpack", [128, 3200], BF16, kind="ExternalInput").ap()
    biasp = nc.dram_tensor("biasp", [128, 6], F32, kind="ExternalInput").ap()
    outT = nc.dram_tensor("outT", [H, BL], F32, kind="ExternalOutput").ap()

    W_OFF = {"wir": 0, "whr": 512, "wiz": 1152, "whz": 1664,
             "win": 2176, "whn": 2688}
    CWM = CW                      # max chunk width (512); f-halves sit at f*CWM
    widths = [256] + [512] * 7 + [256]

    with tile.TileContext(nc) as tc, ExitStack() as ctx:
        const_pool = ctx.enter_context(tc.tile_pool(name="const", bufs=1))
        x_pool = ctx.enter_context(tc.tile_pool(name="x", bufs=3))
        h_pool = ctx.enter_context(tc.tile_pool(name="h", bufs=3))
        hb_pool = ctx.enter_context(tc.tile_pool(name="hb", bufs=3))
        hs_pool = ctx.enter_context(tc.tile_pool(name="hs", bufs=3))
        xr_pool = ctx.enter_context(tc.tile_pool(name="xr", bufs=2))
        z_pool = ctx.enter_context(tc.tile_pool(name="z", bufs=2))
        s_pool = ctx.enter_context(tc.tile_pool(name="s", bufs=2))
        r_pool = ctx.enter_context(tc.tile_pool(name="r", bufs=2))
        n_pool = ctx.enter_context(tc.tile_pool(name="n", bufs=2))
        d_pool = ctx.enter_context(tc.tile_pool(name="d", bufs=2))
        o_pool = ctx.enter_context(tc.tile_pool(name="o", bufs=2))
        ps_pool = ctx.enter_context(tc.tile_pool(name="ps", bufs=4, space="PSUM"))

        # --- constants ---
        bias_t = const_pool.tile([128, 6], F32, tag="biasp", name="bias_t")
        nc.sync.dma_start(out=bias_t[:, :], in_=biasp[:, :])
        wp_t = const_pool.tile([128, 3200], BF16, tag="wpack", name="wp_t")
        nc.sync.dma_start(out=wp_t[:, 0:512], in_=wpack[:, 0:512])
        nc.sync.dma_start(out=wp_t[:, 512:1152], in_=wpack[:, 512:1152])
        nc.sync.dma_start(out=wp_t[:, 1152:3200], in_=wpack[:, 1152:3200])
        wt = {w: [wp_t[:, o + k * 256:o + (k + 1) * 256] for k in range(2)]
              for w, o in W_OFF.items()}
        id_t = wp_t[:, 1024:1152]
        # warm the ACT spline tables during the initial DMA fill
        warm_t = const_pool.tile([128, 6], F32, tag="warm", name="warm_t")
        nc.scalar.activation(warm_t[:, :], bias_t[:, :],
                             mybir.ActivationFunctionType.Sigmoid)
        nc.scalar.activation(warm_t[:, :], bias_t[:, :],
                             mybir.ActivationFunctionType.Tanh)

        def fcols(t, f):
            return t[:, f * 128:(f + 1) * 128]

        pend = None

        def emit_tail(st, last=False):
            c, sl, cw, pn, sc, zt, ht, nt, dt_, ot = st
            for f in range(2):
                ph = pn[:, f * CWM:f * CWM + cw]
                nc.tensor.matmul(ph, fcols(wt["whn"][0], f),
                                 sc[:, 0:cw], start=False, stop=False)
                nc.tensor.matmul(ph, fcols(wt["whn"][1], f),
                                 sc[:, CWM:CWM + cw], start=False, stop=True)
            for f in range(2):
                nc.scalar.activation(nt[:, f * CWM:f * CWM + cw],
                                     pn[:, f * CWM:f * CWM + cw],
                                     mybir.ActivationFunctionType.Tanh,
                                     bias=bias_t[:, f * 3 + 2:f * 3 + 3])

            def v2(t):  # [128, 2, cw] f-half view
                if cw == CWM:
                    return t[:, 0:2 * CWM]
                return t[:, 0:2 * CWM].rearrange(
                    "p (f b) -> p f b", f=2)[:, :, 0:cw]

            # out = n + z * (h - n); GPSIMD does sub+mul in steady state,
            # the DVE does the whole chain on the last chunk (shorter drain)
            if last:
                nc.vector.tensor_sub(v2(dt_), v2(ht), v2(nt))
                nc.vector.tensor_mul(v2(dt_), v2(zt), v2(dt_))
            else:
                nc.gpsimd.tensor_sub(v2(dt_), v2(ht), v2(nt))
                nc.gpsimd.tensor_mul(v2(dt_), v2(zt), v2(dt_))
            nc.vector.tensor_add(v2(ot), v2(nt), v2(dt_))
            ov = ot[:, 0:2 * CWM].rearrange("p (f b) -> p f b", f=2)
            nc.sync.dma_start(
                out=outT[:, sl].rearrange("(f p) b -> p f b", f=2),
                in_=ov if cw == CWM else ov[:, :, 0:cw])

        off = 0
        for c, cw in enumerate(widths):
            sl = slice(off, off + cw)
            off += cw

            # x first (unblocks the xr matmuls), then the wave-1 hs
            # neighbors, then h/hb, then the rest of hs: per-neighbor DMAs
            # give fine-grained completion so wave 1 starts early.
            def dsl(t, w2):  # dest view: flat-rearranged, cw-sliced if fringe
                v = t[:, 0:w2].rearrange("p (f b) -> p f b", f=2)
                return v if cw == CWM else v[:, :, 0:cw]

            xt = x_pool.tile([128, 2 * CWM], BF16, tag="x", name=f"x_{c}")
            nc.sync.dma_start(
                out=dsl(xt, 2 * CWM),
                in_=xT[:, sl].rearrange("(f p) b -> p f b", f=2))
            hsc = hs_pool.tile([128, 2 * N_NEIGH * CWM], BF16, tag="hs",
                               name=f"hs_{c}")

            def hs_dma(n):
                v = hsc[:, n * 2 * CWM:(n + 1) * 2 * CWM].rearrange(
                    "p (f b) -> p f b", f=2)
                nc.sync.dma_start(
                    out=v if cw == CWM else v[:, :, 0:cw],
                    in_=hsT[n, :, sl].rearrange("(f p) b -> p f b", f=2))

            for n in (0, 1, 2):
                hs_dma(n)
            ht = h_pool.tile([128, 2 * CWM], F32, tag="h", name=f"h_{c}")
            nc.sync.dma_start(
                out=dsl(ht, 2 * CWM),
                in_=hT[:, sl].rearrange("(f p) b -> p f b", f=2))
            htb = hb_pool.tile([128, 2 * CWM], BF16, tag="hb", name=f"hb_{c}")
            nc.sync.dma_start(
                out=dsl(htb, 2 * CWM),
                in_=hbT[:, sl].rearrange("(f p) b -> p f b", f=2))
            for n in (3, 4, 5, 6, 7):
                hs_dma(n)

            def blk(t, b0, nb):  # [128, nb, cw] view of 512-wide blocks
                if cw == CWM:
                    return t[:, b0 * CWM:(b0 + nb) * CWM]
                return t[:, b0 * CWM:(b0 + nb) * CWM].rearrange(
                    "p (k b) -> p k b", k=nb)[:, :, 0:cw]

            def hs_k(n, k):  # [128, cw] matmul operand, k-chunk k
                return hsc[:, (n * 2 + k) * CWM:(n * 2 + k) * CWM + cw]

            # --- xr = Wir@x (+ b_r via ACT), bf16 in SBUF; later added into
            #     each neighbor's PSUM group via an identity matmul ---
            pxr = ps_pool.tile([128, 2 * CWM], F32, tag="ps", name=f"pxr_{c}")
            for f in range(2):
                ph = pxr[:, f * CWM:f * CWM + cw]
                nc.tensor.matmul(ph, fcols(wt["wir"][0], f), xt[:, 0:cw],
                                 start=True, stop=False)
                nc.tensor.matmul(ph, fcols(wt["wir"][1], f),
                                 xt[:, CWM:CWM + cw], start=False, stop=True)
            xr = xr_pool.tile([128, 2 * CWM], BF16, tag="xr", name=f"xr_{c}")
            for f in range(2):
                nc.scalar.add(xr[:, f * CWM:f * CWM + cw],
                              pxr[:, f * CWM:f * CWM + cw],
                              bias_t[:, f * 3:f * 3 + 1])
            # previous chunk's tail lands here: its Whn@s matmuls sit in the
            # PE queue behind this chunk's xr group, so the PE stays fed
            # while the DVE finishes the previous product/sum tree
            if pend is not None:
                emit_tail(pend)

            # --- r neighbors in waves of 3; weight-major emission inside a
            #     wave so the stationary operand is reused across neighbors ---
            rc = r_pool.tile([128, 2 * N_NEIGH * CWM], BF16, tag="r",
                             name=f"r_{c}")
            sc = s_pool.tile([128, 2 * CWM], BF16, tag="s", name=f"s_{c}")
            prs = {}
            for wave in ((0, 1, 2), (3, 4, 5), (6, 7)):
                for n in wave:
                    prs[n] = ps_pool.tile([128, 2 * CWM], F32, tag="ps",
                                          name=f"pr{n}_{c}")
                for f in range(2):
                    for k in range(2):
                        for n in wave:
                            nc.tensor.matmul(
                                prs[n][:, f * CWM:f * CWM + cw],
                                fcols(wt["whr"][k], f), hs_k(n, k),
                                start=(k == 0), stop=False)
                # identity injection of xr (+b_r); same stationary for all
                for n in wave:
                    for f in range(2):
                        nc.tensor.matmul(prs[n][:, f * CWM:f * CWM + cw],
                                         id_t[:, :],
                                         xr[:, f * CWM:f * CWM + cw],
                                         start=False, stop=True)
                if wave[0] == 6:
                    # per-half sigmoids: f0 starts under the remaining id
                    # matmuls (subtile deps), shortening the boundary chain
                    for n in wave:
                        for f in range(2):
                            nc.scalar.activation(
                                rc[:, (n * 2 + f) * CWM:(n * 2 + f) * CWM + cw],
                                prs[n][:, f * CWM:f * CWM + cw],
                                mybir.ActivationFunctionType.Sigmoid)
                else:
                    for n in wave:
                        nc.scalar.activation(blk(rc, n * 2, 2),
                                             blk(prs[n], 0, 2),
                                             mybir.ActivationFunctionType.Sigmoid)
                # DVE products for completed sigmoids, in place in hsc;
                # wide block adds; short post-r7 chain (mul6 under r7's ACT)
                with nc.allow_low_precision(reason="bf16 neighbor prod/sums"):
                    if wave[0] == 0:
                        nc.vector.tensor_mul(blk(hsc, 0, 4), blk(rc, 0, 4),
                                             blk(hsc, 0, 4))
                    elif wave[0] == 3:
                        nc.vector.tensor_mul(blk(hsc, 4, 8), blk(rc, 4, 8),
                                             blk(hsc, 4, 8))
                        # (n0,n1) += (n2,n3), then n0 += n1
                        nc.vector.tensor_add(blk(hsc, 0, 4), blk(hsc, 0, 4),
                                             blk(hsc, 4, 4))
                        nc.vector.tensor_add(blk(hsc, 0, 2), blk(hsc, 0, 2),
                                             blk(hsc, 2, 2))
                    else:
                        # fold n4,n5 into n0 early; then per-half products
                        # so the f0 half of s completes first and unblocks
                        # the Whn@s f0 matmuls
                        def b1(t, i):
                            return t[:, i * CWM:i * CWM + cw]

                        nc.vector.tensor_add(blk(hsc, 8, 2), blk(hsc, 8, 2),
                                             blk(hsc, 10, 2))
                        nc.vector.tensor_add(blk(hsc, 0, 2), blk(hsc, 0, 2),
                                             blk(hsc, 8, 2))
                        nc.vector.tensor_mul(b1(hsc, 12), b1(rc, 12),
                                             b1(hsc, 12))
                        nc.vector.tensor_mul(b1(hsc, 13), b1(rc, 13),
                                             b1(hsc, 13))
                        nc.vector.tensor_mul(b1(hsc, 14), b1(rc, 14),
                                             b1(hsc, 14))
                        nc.vector.tensor_add(b1(hsc, 12), b1(hsc, 12),
                                             b1(hsc, 14))
                        nc.vector.tensor_add(sc[:, 0:cw], b1(hsc, 0),
                                             b1(hsc, 12))
                        nc.vector.tensor_mul(b1(hsc, 15), b1(rc, 15),
                                             b1(hsc, 15))
                        nc.vector.tensor_add(b1(hsc, 13), b1(hsc, 13),
                                             b1(hsc, 15))
                        nc.vector.tensor_add(sc[:, CWM:CWM + cw], b1(hsc, 1),
                                             b1(hsc, 13))



            # --- n gate part 1: Win@x into PSUM (keeps PE busy while the
            #     DVE finishes the tree), then the whole z gate; Whn@s and
            #     everything after it are deferred into the next chunk ---
            pn = ps_pool.tile([128, 2 * CWM], F32, tag="ps", name=f"pn_{c}")
            for f in range(2):
                ph = pn[:, f * CWM:f * CWM + cw]
                nc.tensor.matmul(ph, fcols(wt["win"][0], f), xt[:, 0:cw],
                                 start=True, stop=False)
                nc.tensor.matmul(ph, fcols(wt["win"][1], f),
                                 xt[:, CWM:CWM + cw], start=False, stop=False)

            pz = ps_pool.tile([128, 2 * CWM], F32, tag="ps", name=f"pz_{c}")
            for f in range(2):
                ph = pz[:, f * CWM:f * CWM + cw]
                nc.tensor.matmul(ph, fcols(wt["wiz"][0], f), xt[:, 0:cw],
                                 start=True, stop=False)
                nc.tensor.matmul(ph, fcols(wt["wiz"][1], f),
                                 xt[:, CWM:CWM + cw], start=False, stop=False)
                nc.tensor.matmul(ph, fcols(wt["whz"][0], f), htb[:, 0:cw],
                                 start=False, stop=False)
                nc.tensor.matmul(ph, fcols(wt["whz"][1], f),
                                 htb[:, CWM:CWM + cw], start=False, stop=True)
            zt = z_pool.tile([128, 2 * CWM], F32, tag="z", name=f"z_{c}")
            for f in range(2):
                nc.scalar.activation(zt[:, f * CWM:f * CWM + cw],
                                     pz[:, f * CWM:f * CWM + cw],
                                     mybir.ActivationFunctionType.Sigmoid,
                                     bias=bias_t[:, f * 3 + 1:f * 3 + 2])

            nt = n_pool.tile([128, 2 * CWM], F32, tag="n", name=f"n_{c}")
            dt_ = d_pool.tile([128, 2 * CWM], F32, tag="d", name=f"d_{c}")
            ot = o_pool.tile([128, 2 * CWM], F32, tag="o", name=f"o_{c}")
            pend = (c, sl, cw, pn, sc, zt, ht, nt, dt_, ot)

        emit_tail(pend, last=True)

    nc.compile()
    return nc


def _prep_inputs(x, h_sum, hs, Wir, bir, Whr, bhr, Wiz, biz, Whz, bhz,
                 Win, bin_, Whn, bhn):
    """Shard + transpose to feature-major per-core input maps."""
    f32 = np.float32
    xT = np.asarray(x, f32).T.astype(BF_NP)                  # [256, B] bf16
    hT = np.ascontiguousarray(np.asarray(h_sum, f32).T)      # [256, B] f32
    hbT = hT.astype(BF_NP)                                   # [256, B] bf16
    hsT = np.asarray(hs, f32).transpose(0, 2, 1).astype(BF_NP)  # [8,256,B] bf16

    wpack = np.empty((128, 3200), BF_NP)
    offs = {"wir": 0, "whr": 512, "wiz": 1152, "whz": 1664,
            "win": 2176, "whn": 2688}
    mats = {"wir": Wir, "whr": Whr, "wiz": Wiz, "whz": Whz,
            "win": Win, "whn": Whn}
    for wname, o in offs.items():
        wT = np.asarray(mats[wname], f32).T.astype(BF_NP)   # [256 in, 256 out]
        for k in range(2):
            wpack[:, o + k * 256:o + (k + 1) * 256] = \
                wT[k * 128:(k + 1) * 128, :]
    wpack[:, 1024:1152] = np.eye(128, dtype=f32).astype(BF_NP)
    b_r = np.asarray(bir, f32) + np.asarray(bhr, f32)
    b_z = np.asarray(biz, f32) + np.asarray(bhz, f32)
    b_n = np.asarray(bin_, f32) + np.asarray(bhn, f32)
    biasp = np.empty((128, 6), f32)
    for f in range(2):
        biasp[:, f * 3 + 0] = b_r[f * 128:(f + 1) * 128]
        biasp[:, f * 3 + 1] = b_z[f * 128:(f + 1) * 128]
        biasp[:, f * 3 + 2] = b_n[f * 128:(f + 1) * 128]

    in_maps = []
    for c in range(M):
        sl = slice(c * BL, (c + 1) * BL)
        m = {
            "xT": np.ascontiguousarray(xT[:, sl]),
            "hT": np.ascontiguousarray(hT[:, sl]),
            "hbT": np.ascontiguousarray(hbT[:, sl]),
            "hsT": np.ascontiguousarray(hsT[:, :, sl]),
            "wpack": wpack,
            "biasp": biasp,
        }
        in_maps.append(m)
    return in_maps


def _run(inputs, trace=False, **trace_kwargs):
    global _cached
    if _cached is None:
        _cached = _build()
    nc = _cached
    in_maps = _prep_inputs(**inputs)
    res = run_bass_kernel_spmd(nc, in_maps, list(range(M)), trace=trace,
                               **trace_kwargs)
    out = np.empty((B, H), np.float32)
    for c in range(M):
        out[c * BL:(c + 1) * BL, :] = res.results[c]["outT"].T
    return out, res


def kernel(**inputs):
    return _run(inputs)[0]
